# revision 34
# baseline (speedup 1.0000x reference)
"""Trainium2 Bass kernel for gated-attention pooling (nn_AttentionGated).

Computation (reference):
    h = relu(x[0] @ W_feat.T + b_feat)        # [N, 768]
    a = relu(h @ W_a.T)                        # [N, 128]
    b = sigmoid(h @ W_b.T)                     # [N, 128]
    logits = (a*b) @ W_c.T                     # [N] -> softmax over N
    out = softmax(logits) @ h                  # [1, 768]

Strategy: shard N=50000 rows over 8 cores (6250 each, padded to 13 blocks of
512 rows). Everything stays in TRANSPOSED [feature-on-partition, row-on-free]
layout, which removes the baseline's PE transposes, bias matmuls and PSUM
transpose-evacuation entirely:

  hT[e, n] = relu(16*W_feat @ x^T + 16*b): 18 fp8 DoubleRow MMs per block
      (stationary = W chunks, moving = x^T chunks). The bias rides the
      PSUM->SBUF evacuations as a per-partition scalar (e is the partition
      axis here): ACT relu(psum + bias) or DVE (psum add bias) max 0, cast
      straight to fp8 (16h fits e4m3 comfortably).
  aT,bT = 256*(W_{a,b} @ h): 6 more DR MMs on the fp8 hT, lagged one block
      so the evacuations stay off the PE critical path.
  sigmoid without ACT-table switches (sigmoid and exp never share an ACT
      table; reloads cost 1.3us each): sigmoid(z) = 0.5*(1+tanh(z/2)), and
      TANH co-resides with Exp/Relu/Identity in the exp_and_others table:
      th = tanh(bT/512) [ACT], v = 0.5*th+0.5 [DVE tensor_scalar, 4x mode],
      gw = relu(aT)*v [DVE scalar_tensor_tensor from PSUM]. ACT then never
      reloads its table.
  logits = one MM with stationary (W_c/256 replicated over M) against gw:
      out[m, n] = logit_n for every m -- BROADCAST logits [128, 512] written
      back into the aT PSUM bank (free after gw consumed it), so
      w = exp(logits) [ACT, accum_out = partial softmax denominator Z for
      free] is already replicated across partitions for the P stage.
  P[e] += sum_n hT[e, n]*w[n]: scalar_tensor_tensor with accum_out on DVE
      (no DVE fast mode exists for this op, so it runs at ~1 col/cycle
      regardless of dtype), batched over 2-block pairs for the steady-state
      positions and unpaired for the last four so the pipeline drain stays
      short; slab columns are reduced once at the end on ACT (Identity with
      accum_out) while the DVE drains.

The host merges the 8 partial (P, Z) pairs: out = sum(P_i)/16 / sum(Z_i).
No on-device collective. W_feat/W_ab/b are pre-scaled x16 on the host to
dodge fp8e4 subnormals; hT is stored as 16h, so aT/bT come out x256 (undone
by the exp scale and by W_c/256) and P comes out x16 (undone on the host).
"""

import sys
import types

import numpy as np
import ml_dtypes

import concourse.bass as bass
import concourse.bacc as bacc
import concourse.mybir as mybir
from concourse import tile
from concourse.bass_utils import run_bass_kernel_spmd

BF16 = ml_dtypes.bfloat16
FP8 = ml_dtypes.float8_e4m3
W_SCALE = 16.0

N_CORES = 8
N = 50000
DIM = 768
D_ATT = 128
NS = N // N_CORES            # 6250 rows per core
BLK = 512                    # rows per block (one full PSUM bank of fp32)
NB = 13                      # blocks per core (6656 rows, last 406 padded)
NRED = 7                     # 6 pairs + 1 solo narrow block
WL = 128                     # width of the narrow last block
LAST_VALID = NS - (NB - 1) * BLK  # 106 valid rows in the last block

_cached_nc = None
last_results = None  # BassKernelResults of the most recent run (for profiling)


def _build_nc():
    AF = mybir.ActivationFunctionType
    ALU = mybir.AluOpType
    dt = mybir.dt

    nc = bacc.Bacc("TRN2", target_bir_lowering=False, debug=False)

    xt_d = nc.dram_tensor("xt", [NB, 128, 3, 2, BLK], dt.float8e4, kind="ExternalInput").ap()
    wt_d = nc.dram_tensor("wt", [128, 6, 3, 2, 128], dt.float8e4, kind="ExternalInput").ap()
    wab_d = nc.dram_tensor("wab", [128, 2, 3, 2, 128], dt.float8e4, kind="ExternalInput").ap()
    bcol_d = nc.dram_tensor("bcol", [128, 6], dt.float32, kind="ExternalInput").ap()
    wcones_d = nc.dram_tensor("wcones", [128, 128], dt.bfloat16, kind="ExternalInput").ap()
    mask_d = nc.dram_tensor("mask", [128, WL], dt.bfloat16, kind="ExternalInput").ap()
    out_d = nc.dram_tensor("out", [128, 8], dt.float32, kind="ExternalOutput").ap()

    # evacuation engine per e-chunk: ACT is cheaper per column and the DVE
    # carries the P accumulations, so ACT takes five of the six.
    EVAC = ["scalar", "vector", "scalar", "scalar", "scalar", "scalar"]

    with tile.TileContext(nc) as tc:
        with (
            tc.tile_pool(name="const", bufs=1) as constp,
            tc.tile_pool(name="xtp", bufs=3) as xtp,
            tc.tile_pool(name="hp", bufs=4) as hp,
            tc.tile_pool(name="tp", bufs=2) as tp,
            tc.tile_pool(name="sgp", bufs=2) as sgp,
            tc.tile_pool(name="gwp", bufs=2) as gwp,
            tc.tile_pool(name="wp", bufs=2) as wp,
            tc.tile_pool(name="trd", bufs=2) as trdp,
            tc.tile_pool(name="psA", bufs=4, space="PSUM") as psA,
            tc.tile_pool(name="psB", bufs=2, space="PSUM") as psB,
        ):
            # --- constants (loaded once) ---
            wt_sb = constp.tile([128, 6, 3, 2, 128], dt.float8e4)
            nc.sync.dma_start(wt_sb[:, 0:3], wt_d[:, 0:3])
            nc.scalar.dma_start(wt_sb[:, 3:6], wt_d[:, 3:6])
            wab_sb = constp.tile([128, 2, 3, 2, 128], dt.float8e4)
            nc.scalar.dma_start(wab_sb[:], wab_d[:])
            bcol_sb = constp.tile([128, 6], dt.float32)
            nc.scalar.dma_start(bcol_sb[:], bcol_d[:])
            wcones_sb = constp.tile([128, 128], dt.bfloat16)
            nc.scalar.dma_start(wcones_sb[:], wcones_d[:])
            mask_sb = constp.tile([128, 128], dt.bfloat16)
            nc.scalar.dma_start(mask_sb[:], mask_d[:])

            zslab = constp.tile([128, 16], dt.float32)
            pslab = constp.tile([128, 6, 12], dt.float32)
            out_sb = constp.tile([128, 8], dt.float32)

            # Blocks run in natural order with the NARROW block (12) LAST:
            # the pipeline drain then ends on the cheap 128-wide P/sigma
            # chain instead of a full 512-wide pair. Full blocks pair up as
            # (0,1),(2,3),... for the batched P accumulation.
            ORDER = list(range(NB))
            hts = [None] * (NB + 1)    # per position: (pair_tile, slot)
            psbs = [None] * (NB + 1)
            w2s = [None] * (NB + 1)
            wid = lambda pos: WL if pos == NPOS - 1 else BLK

            def pair_of(pos):
                # pairs (0,1)..(10,11) -> 0..5; solo narrow position 12 -> 6
                return 6 if pos == NPOS - 1 else pos // 2

            def p_acc(pos, c):
                """P accumulation for the pair ending at even position pos
                (chunks summed over both blocks), or the solo position 0."""
                ht2 = hts[pos][0]
                w2, slot = w2s[pos]
                trd = trdp.tile([128, 2, BLK], dt.bfloat16, tag="trd")
                pair = pair_of(pos)
                if pos == NPOS - 1:
                    trd = trd[:, 0, 0:WL]
                    srcp = ht2[:, c // 2, c % 2, 0, 0:WL]
                    wsrc = w2[:, 0, 0:WL]
                else:
                    trd = trd[:]
                    srcp = ht2[:, c // 2, c % 2, :, :]
                    wsrc = w2[:]
                nc.vector.scalar_tensor_tensor(
                    trd, srcp, 0.0, wsrc, ALU.bypass, ALU.mult,
                    accum_out=pslab[:, c, pair:pair + 1])

            NPOS = NB
            # Stage schedules: position 0 (the narrow block) runs with
            # tighter lags (ab at it=0, sigma at it=1, w at it=2) to pull
            # the whole pipeline one period earlier; the rest use the
            # steady-state lags ab=it-1, sigma=it-2, w=it-3.
            SCHED_AB = {0: 0}
            SCHED_SG = {1: 0}
            SCHED_W = {2: 0}
            for _it in range(2, NPOS + 5):
                if 1 <= _it - 1 < NPOS:
                    SCHED_AB[_it] = _it - 1
            for _it in range(3, NPOS + 5):
                if 1 <= _it - 2 < NPOS:
                    SCHED_SG[_it] = _it - 2
            for _it in range(4, NPOS + 5):
                if 1 <= _it - 3 < NPOS:
                    SCHED_W[_it] = _it - 3
            # P-accumulation triggers: {it: [(pos, chunks)]}
            SCHED_P = {}
            for _pos in range(1, NPOS - 1, 2):   # pairs complete at odd pos
                SCHED_P.setdefault(_pos + 3, []).append((_pos, (0, 1, 2)))
                SCHED_P.setdefault(_pos + 4, []).append((_pos, (3, 4, 5)))
            SCHED_P.setdefault(NPOS + 2, []).append((NPOS - 1, (0, 1, 2)))
            SCHED_P.setdefault(NPOS + 3, []).append((NPOS - 1, (3, 4, 5)))

            for it in range(NPOS + 5):
                pw = SCHED_W.get(it)
                pl = SCHED_SG.get(it)
                pa = SCHED_AB.get(it)

                # ---- ACT: th(pl) first (feeds the v/gw/logit chain) ----
                if pl is not None:
                    wj = wid(pl)
                    t_sb = tp.tile([128, BLK], dt.bfloat16, tag="t")
                    t_sb = t_sb[:, 0:wj]
                    nc.scalar.activation(t_sb, psbs[pl][:, 1, 0:wj], AF.Tanh,
                                         scale=1.0 / 512.0)

                def w_stage():
                    if pw is None:
                        return
                    blk = ORDER[pw]
                    if pw % 2 == 0:
                        w2 = wp.tile([128, 2, BLK], dt.bfloat16, tag="w")
                        slot = 0
                    else:
                        w2 = w2s[pw - 1][0]
                        slot = 1
                    w2s[pw] = (w2, slot)
                    if pw == NPOS - 1:
                        # narrow block: mask the padded rows out of w and
                        # compute Z with an explicit masked reduce.
                        wdst = w2[:, 0, 0:WL]
                        wraw = wp.tile([128, BLK], dt.bfloat16, tag="wr")
                        nc.scalar.activation(wraw[:, 0:WL],
                                             psbs[pw][:, 0, 0:WL], AF.Exp)
                        nc.vector.tensor_tensor(wdst, wraw[:, 0:WL],
                                                mask_sb[:], ALU.mult)
                        nc.vector.tensor_reduce(zslab[:, blk:blk + 1], wdst,
                                                mybir.AxisListType.X, ALU.add)
                    else:
                        wdst = w2[:, slot, :]
                        nc.scalar.activation(wdst, psbs[pw][:, 0], AF.Exp,
                                             accum_out=zslab[:, blk:blk + 1])

                # ---- PE + evacs: h GEMM for position it ----
                if it < NPOS:
                    blk = ORDER[it]
                    wj = wid(it)
                    xt = xtp.tile([128, 3, 2, BLK], dt.float8e4, tag="xt")
                    xt = xt[:, :, :, 0:wj]
                    nc.sync.dma_start(xt[:], xt_d[blk, :, :, :, 0:wj])

                    if it % 2 == 0:
                        ht2 = hp.tile([128, 3, 2, 2, BLK], dt.float8e4,
                                      tag="ht")
                        slot = 0
                    else:
                        ht2 = hts[it - 1][0]
                        slot = 1
                    hts[it] = (ht2, slot)
                    for c in range(6):
                        psa = psA.tile([128, BLK], dt.float32, tag="psa")
                        psa = psa[:, 0:wj]
                        for s in range(3):
                            nc.tensor.matmul(
                                psa[:], wt_sb[:, c, s], xt[:, s],
                                start=(s == 0), stop=(s == 2),
                                perf_mode=mybir.MatmulPerfMode.DoubleRow,
                                skip_group_check=True)
                        dst = ht2[:, c // 2, c % 2, slot, 0:wj]
                        bias = bcol_sb[:, c:c + 1]
                        if EVAC[c] == "scalar":
                            nc.scalar.activation(dst, psa, AF.Relu,
                                                 bias=bias)
                        else:
                            nc.vector.tensor_scalar(dst, psa, bias, 0.0,
                                                    ALU.add, ALU.max)
                        if c == 0:
                            # w emitted after the first evacuation so the
                            # psA rotation is fed promptly
                            w_stage()
                else:
                    w_stage()

                # ---- DVE: sigma chain for pl ----
                if pl is not None:
                    wj = wid(pl)
                    v_sb = sgp.tile([128, BLK], dt.bfloat16, tag="sg")
                    v_sb = v_sb[:, 0:wj]
                    nc.vector.tensor_scalar(v_sb, t_sb, 0.5, 0.5,
                                            ALU.mult, ALU.add)
                    gw_sb = gwp.tile([128, BLK], dt.bfloat16, tag="gw")
                    gw_sb = gw_sb[:, 0:wj]
                    nc.vector.scalar_tensor_tensor(
                        gw_sb, psbs[pl][:, 0, 0:wj], 0.0, v_sb,
                        ALU.max, ALU.mult)

                # ---- DVE: P accumulations per schedule ----
                for pos_, chunks in SCHED_P.get(it, ()):
                    for c in chunks:
                        p_acc(pos_, c)

                # ---- PE: ab GEMM for position pa ----
                if pa is not None:
                    wj = wid(pa)
                    psb = psB.tile([128, 2, BLK], dt.float32, tag="psb")
                    psbs[pa] = psb
                    ht2, slot = hts[pa]
                    for g in range(2):
                        for s in range(3):
                            nc.tensor.matmul(
                                psb[:, g, 0:wj], wab_sb[:, g, s],
                                ht2[:, s, :, slot, 0:wj],
                                start=(s == 0), stop=(s == 2),
                                perf_mode=mybir.MatmulPerfMode.DoubleRow,
                                skip_group_check=True)

                # ---- PE: logits for pl (aT PSUM bank recycled) ----
                if pl is not None:
                    wj = wid(pl)
                    nc.tensor.matmul(psbs[pl][:, 0, 0:wj], wcones_sb[:],
                                     gw_sb, start=True, stop=True,
                                     skip_group_check=True)

            # ---- finalize: reduce slabs (ACT, overlapping the DVE
            # drain), z on DVE, write out ----
            rtrash = constp.tile([128, 6, NRED], dt.float32)
            for c in range(6):
                nc.scalar.activation(rtrash[:, c], pslab[:, c, 0:NRED],
                                     AF.Identity,
                                     accum_out=out_sb[:, c:c + 1])
            nc.vector.tensor_reduce(out_sb[:, 6:7], zslab[:, 0:NB],
                                    mybir.AxisListType.X, ALU.add)
            nc.vector.memset(out_sb[:, 7:8], 0.0)
            nc.sync.dma_start(out_d[:], out_sb[:])

    nc.compile()
    return nc


def get_nc():
    global _cached_nc
    if _cached_nc is None:
        _cached_nc = _build_nc()
    return _cached_nc


def make_inputs(x, W_feat, b_feat, W_a, W_b, W_c):
    """Host-side preprocessing: shard + retile x into the transposed
    DoubleRow layout, prepack weights."""
    x = np.asarray(x, dtype=np.float32)
    xs = x.reshape(N, DIM)
    NP = NB * BLK
    xp = np.zeros((N_CORES, NP, DIM), dtype=np.float32)
    xp[:, :NS, :] = xs.reshape(N_CORES, NS, DIM)
    # xt[core, j, p, s, i, n] = x[core, j*BLK + n, d=(2s+i)*128 + p]
    blocks = xp.reshape(N_CORES, NB, BLK, 3, 2, 128)   # [r, j, n, s, i, p]
    xt_host = np.ascontiguousarray(blocks.transpose(0, 1, 5, 3, 4, 2)) \
        .astype(FP8)

    WT = np.asarray(W_feat, np.float32) * W_SCALE      # [e, d]
    # wt[p, c, s, i, m] = 16*W_feat[c*128+m, (2s+i)*128+p]
    wt_host = np.ascontiguousarray(
        WT.reshape(6, 128, 3, 2, 128).transpose(4, 0, 2, 3, 1)).astype(FP8)

    wab = np.stack([np.asarray(W_a, np.float32),
                    np.asarray(W_b, np.float32)]) * W_SCALE  # [2, k, e]
    # wab[p, g, s, i, m] = 16*W_g[m, (2s+i)*128+p]
    wab_host = np.ascontiguousarray(
        wab.reshape(2, 128, 3, 2, 128).transpose(4, 0, 2, 3, 1)).astype(FP8)

    # bcol[p, c] = 16*b_feat[c*128+p]
    bcol_host = np.ascontiguousarray(
        (np.asarray(b_feat, np.float32).reshape(6, 128).T * W_SCALE)
        .astype(np.float32))

    wcones_host = np.ascontiguousarray(np.tile(
        np.asarray(W_c, np.float32).reshape(D_ATT, 1) / 256.0,
        (1, 128))).astype(BF16)

    mask_host = np.zeros((128, WL), dtype=BF16)
    mask_host[:, :LAST_VALID] = 1

    common = dict(wt=wt_host, wab=wab_host, bcol=bcol_host,
                  wcones=wcones_host, mask=mask_host)
    return [dict(xt=np.ascontiguousarray(xt_host[i]), **common)
            for i in range(N_CORES)]


def _ensure_axon_profile_hook():
    """If someone runs kernel() with BASS_TRACE=1 under axon, the spmd runner
    imports antenv.axon_hooks, which this image lacks; shim it from
    trn_agent_boot so tracing degrades gracefully instead of crashing."""
    try:
        import antenv.axon_hooks  # noqa: F401
        return
    except ImportError:
        pass
    try:
        from trn_agent_boot import trn_boot

        hook = trn_boot._ntff_profile_via_ctypes("/opt/axon/libaxon_pjrt.so")
        mod = types.ModuleType("antenv.axon_hooks")
        mod.get_axon_ntff_profile_hook = lambda: hook
        mod.set_axon_ntff_profile_hook = lambda h: None
        sys.modules["antenv.axon_hooks"] = mod
    except Exception:
        pass


def kernel(x, W_feat, b_feat, W_a, W_b, W_c):
    global last_results
    _ensure_axon_profile_hook()
    nc = get_nc()
    in_maps = make_inputs(x, W_feat, b_feat, W_a, W_b, W_c)
    res = run_bass_kernel_spmd(nc, in_maps, core_ids=list(range(N_CORES)))
    last_results = res
    P = np.zeros(DIM, dtype=np.float64)
    Z = 0.0
    for r in res.results:
        o = np.asarray(r["out"], dtype=np.float64)   # [128, 8]
        P += o[:, 0:6].T.reshape(DIM)
        Z += o[0, 6]
    return (P / W_SCALE / Z).astype(np.float32).reshape(1, DIM)


# revision 35
# speedup vs baseline: 1.1903x; 1.1903x over previous
"""Trainium2 Bass kernel for gated-attention pooling (nn_AttentionGated).

Computation (reference):
    h = relu(x[0] @ W_feat.T + b_feat)        # [N, 768]
    a = relu(h @ W_a.T)                        # [N, 128]
    b = sigmoid(h @ W_b.T)                     # [N, 128]
    logits = (a*b) @ W_c.T                     # [N] -> softmax over N
    out = softmax(logits) @ h                  # [1, 768]

Strategy: shard N=50000 rows over 8 cores (6250 each, padded to 13 blocks of
512 rows). Everything stays in TRANSPOSED [feature-on-partition, row-on-free]
layout, which removes the baseline's PE transposes, bias matmuls and PSUM
transpose-evacuation entirely:

  hT[e, n] = relu(16*W_feat @ x^T + 16*b): 18 fp8 DoubleRow MMs per block
      (stationary = W chunks, moving = x^T chunks). The bias rides the
      PSUM->SBUF evacuations as a per-partition scalar (e is the partition
      axis here): ACT relu(psum + bias) or DVE (psum add bias) max 0, cast
      straight to fp8 (16h fits e4m3 comfortably).
  aT,bT = 256*(W_{a,b} @ h): 6 more DR MMs on the fp8 hT, lagged one block
      so the evacuations stay off the PE critical path.
  sigmoid without ACT-table switches (sigmoid and exp never share an ACT
      table; reloads cost 1.3us each): sigmoid(z) = 0.5*(1+tanh(z/2)), and
      TANH co-resides with Exp/Relu/Identity in the exp_and_others table:
      th = tanh(bT/512) [ACT], v = 0.5*th+0.5 [DVE tensor_scalar, 4x mode],
      gw = relu(aT)*v [DVE scalar_tensor_tensor from PSUM]. ACT then never
      reloads its table.
  logits = one MM with stationary (W_c/256 replicated over M) against gw:
      out[m, n] = logit_n for every m -- BROADCAST logits [128, 512] written
      back into the aT PSUM bank (free after gw consumed it), so
      w = exp(logits) [ACT, accum_out = partial softmax denominator Z for
      free] is already replicated across partitions for the P stage.
  P[e] += sum_n hT[e, n]*w[n]: scalar_tensor_tensor with accum_out on DVE
      (no DVE fast mode exists for this op, so it runs at ~1 col/cycle
      regardless of dtype), batched over 2-block pairs for the steady-state
      positions and unpaired for the last four so the pipeline drain stays
      short; slab columns are reduced once at the end on ACT (Identity with
      accum_out) while the DVE drains.

The host merges the 8 partial (P, Z) pairs: out = sum(P_i)/16 / sum(Z_i).
No on-device collective. W_feat/W_ab/b are pre-scaled x16 on the host to
dodge fp8e4 subnormals; hT is stored as 16h, so aT/bT come out x256 (undone
by the exp scale and by W_c/256) and P comes out x16 (undone on the host).
"""

import sys
import types

import numpy as np
import ml_dtypes

import concourse.bass as bass
import concourse.bacc as bacc
import concourse.mybir as mybir
from concourse import tile
from concourse.bass_utils import run_bass_kernel_spmd

BF16 = ml_dtypes.bfloat16
FP8 = ml_dtypes.float8_e4m3
W_SCALE = 16.0

N_CORES = 8
N = 50000
DIM = 768
D_ATT = 128
NS = N // N_CORES            # 6250 rows per core
BLK = 512                    # rows per block (one full PSUM bank of fp32)
NB = 13                      # blocks per core (6656 rows, last 406 padded)
NRED = 7                     # 6 pairs + 1 solo narrow block
WL = 128                     # width of the narrow last block
LAST_VALID = NS - (NB - 1) * BLK  # 106 valid rows in the last block

_cached_nc = None
last_results = None  # BassKernelResults of the most recent run (for profiling)


def _build_nc():
    AF = mybir.ActivationFunctionType
    ALU = mybir.AluOpType
    dt = mybir.dt

    nc = bacc.Bacc("TRN2", target_bir_lowering=False, debug=False)

    xt_d = nc.dram_tensor("xt", [NB, 128, 3, 2, BLK], dt.float8e4, kind="ExternalInput").ap()
    wt_d = nc.dram_tensor("wt", [128, 6, 3, 2, 128], dt.float8e4, kind="ExternalInput").ap()
    wab_d = nc.dram_tensor("wab", [128, 2, 3, 2, 128], dt.float8e4, kind="ExternalInput").ap()
    bcol_d = nc.dram_tensor("bcol", [128, 6], dt.float32, kind="ExternalInput").ap()
    wcones_d = nc.dram_tensor("wcones", [128, 128], dt.bfloat16, kind="ExternalInput").ap()
    mask_d = nc.dram_tensor("mask", [128, WL], dt.bfloat16, kind="ExternalInput").ap()
    out_d = nc.dram_tensor("out", [128, 8], dt.float32, kind="ExternalOutput").ap()

    # evacuation engine per e-chunk: ACT is cheaper per column and the DVE
    # carries the P accumulations, so ACT takes five of the six.
    EVAC = ["scalar", "vector", "scalar", "scalar", "scalar", "scalar"]

    with tile.TileContext(nc) as tc:
        with (
            tc.tile_pool(name="const", bufs=1) as constp,
            tc.tile_pool(name="xtp", bufs=3) as xtp,
            tc.tile_pool(name="hp", bufs=4) as hp,
            tc.tile_pool(name="tp", bufs=2) as tp,
            tc.tile_pool(name="sgp", bufs=2) as sgp,
            tc.tile_pool(name="gwp", bufs=2) as gwp,
            tc.tile_pool(name="wp", bufs=2) as wp,
            tc.tile_pool(name="trd", bufs=2) as trdp,
            tc.tile_pool(name="psA", bufs=4, space="PSUM") as psA,
            tc.tile_pool(name="psB", bufs=2, space="PSUM") as psB,
        ):
            # --- constants (loaded once) ---
            wt_sb = constp.tile([128, 6, 3, 2, 128], dt.float8e4)
            nc.sync.dma_start(wt_sb[:, 0:3], wt_d[:, 0:3])
            nc.scalar.dma_start(wt_sb[:, 3:6], wt_d[:, 3:6])
            wab_sb = constp.tile([128, 2, 3, 2, 128], dt.float8e4)
            nc.scalar.dma_start(wab_sb[:], wab_d[:])
            bcol_sb = constp.tile([128, 6], dt.float32)
            nc.scalar.dma_start(bcol_sb[:], bcol_d[:])
            wcones_sb = constp.tile([128, 128], dt.bfloat16)
            nc.scalar.dma_start(wcones_sb[:], wcones_d[:])
            mask_sb = constp.tile([128, 128], dt.bfloat16)
            nc.scalar.dma_start(mask_sb[:], mask_d[:])

            zslab = constp.tile([128, 16], dt.float32)
            pslab = constp.tile([128, 6, 12], dt.float32)
            out_sb = constp.tile([128, 8], dt.float32)

            # Block processing order: the narrow block (12) goes FIRST so
            # its serial sigma/w/P chain overlaps the steady state instead
            # of forming a long tail. Positions 1..12 hold blocks 0..11 and
            # pair up as (1,2),(3,4),... for the batched P accumulation.
            ORDER = [NB - 1] + list(range(NB - 1))
            hts = [None] * (NB + 1)    # per position: (pair_tile, slot)
            psbs = [None] * (NB + 1)
            w2s = [None] * (NB + 1)
            wid = lambda pos: WL if pos == 0 else BLK

            def pair_of(pos):
                # pairs (1,2)..(11,12) -> 0..5; solo narrow position 0 -> 6
                return 6 if pos == 0 else (pos - 1) // 2

            def p_acc(pos, c):
                """P accumulation for the pair ending at even position pos
                (chunks summed over both blocks), or the solo position 0."""
                ht2 = hts[pos][0]
                w2, slot = w2s[pos]
                trd = trdp.tile([128, 2, BLK], dt.bfloat16, tag="trd")
                pair = pair_of(pos)
                if pos == 0:
                    trd = trd[:, 0, 0:WL]
                    srcp = ht2[:, c // 2, c % 2, 0, 0:WL]
                    wsrc = w2[:, 0, 0:WL]
                else:
                    trd = trd[:]
                    srcp = ht2[:, c // 2, c % 2, :, :]
                    wsrc = w2[:]
                nc.vector.scalar_tensor_tensor(
                    trd, srcp, 0.0, wsrc, ALU.bypass, ALU.mult,
                    accum_out=pslab[:, c, pair:pair + 1])

            NPOS = NB
            # Stage schedules: position 0 (the narrow block) runs with
            # tighter lags (ab at it=0, sigma at it=1, w at it=2) to pull
            # the whole pipeline one period earlier; the rest use the
            # steady-state lags ab=it-1, sigma=it-2, w=it-3.
            SCHED_AB = {0: 0}
            SCHED_SG = {1: 0}
            SCHED_W = {2: 0}
            for _it in range(2, NPOS + 5):
                if 1 <= _it - 1 < NPOS:
                    SCHED_AB[_it] = _it - 1
            for _it in range(3, NPOS + 5):
                if 1 <= _it - 2 < NPOS:
                    SCHED_SG[_it] = _it - 2
            for _it in range(4, NPOS + 5):
                if 1 <= _it - 3 < NPOS:
                    SCHED_W[_it] = _it - 3
            # P-accumulation triggers: {it: [(pos, chunks)]}
            SCHED_P = {2: [(0, (0, 1, 2))], 3: [(0, (3, 4, 5))]}
            for _pos in range(2, NPOS, 2):   # pairs complete at even pos
                SCHED_P.setdefault(_pos + 3, []).append((_pos, (0, 1, 2)))
                SCHED_P.setdefault(_pos + 4, []).append((_pos, (3, 4, 5)))

            for it in range(NPOS + 5):
                pw = SCHED_W.get(it)
                pl = SCHED_SG.get(it)
                pa = SCHED_AB.get(it)

                # ---- ACT: th(pl) first (feeds the v/gw/logit chain) ----
                if pl is not None:
                    wj = wid(pl)
                    t_sb = tp.tile([128, BLK], dt.bfloat16, tag="t")
                    t_sb = t_sb[:, 0:wj]
                    nc.scalar.activation(t_sb, psbs[pl][:, 1, 0:wj], AF.Tanh,
                                         scale=1.0 / 512.0)

                def w_stage():
                    if pw is None:
                        return
                    blk = ORDER[pw]
                    if pw == 0 or pw % 2 == 1:
                        w2 = wp.tile([128, 2, BLK], dt.bfloat16, tag="w")
                        slot = 0
                    else:
                        w2 = w2s[pw - 1][0]
                        slot = 1
                    w2s[pw] = (w2, slot)
                    if pw == 0:
                        # narrow block: mask the padded rows out of w and
                        # compute Z with an explicit masked reduce.
                        wdst = w2[:, 0, 0:WL]
                        wraw = wp.tile([128, BLK], dt.bfloat16, tag="wr")
                        nc.scalar.activation(wraw[:, 0:WL],
                                             psbs[pw][:, 0, 0:WL], AF.Exp)
                        nc.vector.tensor_tensor(wdst, wraw[:, 0:WL],
                                                mask_sb[:], ALU.mult)
                        nc.vector.tensor_reduce(zslab[:, blk:blk + 1], wdst,
                                                mybir.AxisListType.X, ALU.add)
                    else:
                        wdst = w2[:, slot, :]
                        nc.scalar.activation(wdst, psbs[pw][:, 0], AF.Exp,
                                             accum_out=zslab[:, blk:blk + 1])

                # ---- PE + evacs: h GEMM for position it ----
                if it < NPOS:
                    blk = ORDER[it]
                    wj = wid(it)
                    xt = xtp.tile([128, 3, 2, BLK], dt.float8e4, tag="xt")
                    xt = xt[:, :, :, 0:wj]
                    nc.sync.dma_start(xt[:], xt_d[blk, :, :, :, 0:wj])

                    if it == 0 or it % 2 == 1:
                        ht2 = hp.tile([128, 3, 2, 2, BLK], dt.float8e4,
                                      tag="ht")
                        slot = 0
                    else:
                        ht2 = hts[it - 1][0]
                        slot = 1
                    hts[it] = (ht2, slot)
                    for c in range(6):
                        psa = psA.tile([128, BLK], dt.float32, tag="psa")
                        psa = psa[:, 0:wj]
                        for s in range(3):
                            nc.tensor.matmul(
                                psa[:], wt_sb[:, c, s], xt[:, s],
                                start=(s == 0), stop=(s == 2),
                                perf_mode=mybir.MatmulPerfMode.DoubleRow,
                                skip_group_check=True)
                        dst = ht2[:, c // 2, c % 2, slot, 0:wj]
                        bias = bcol_sb[:, c:c + 1]
                        if EVAC[c] == "scalar":
                            nc.scalar.activation(dst, psa, AF.Relu,
                                                 bias=bias)
                        else:
                            nc.vector.tensor_scalar(dst, psa, bias, 0.0,
                                                    ALU.add, ALU.max)
                        if c == 0:
                            # w emitted after the first evacuation so the
                            # psA rotation is fed promptly
                            w_stage()
                else:
                    w_stage()

                # ---- DVE: sigma chain for pl ----
                if pl is not None:
                    wj = wid(pl)
                    v_sb = sgp.tile([128, BLK], dt.bfloat16, tag="sg")
                    v_sb = v_sb[:, 0:wj]
                    nc.vector.tensor_scalar(v_sb, t_sb, 0.5, 0.5,
                                            ALU.mult, ALU.add)
                    gw_sb = gwp.tile([128, BLK], dt.bfloat16, tag="gw")
                    gw_sb = gw_sb[:, 0:wj]
                    nc.vector.scalar_tensor_tensor(
                        gw_sb, psbs[pl][:, 0, 0:wj], 0.0, v_sb,
                        ALU.max, ALU.mult)

                # ---- DVE: P accumulations per schedule ----
                for pos_, chunks in SCHED_P.get(it, ()):
                    for c in chunks:
                        p_acc(pos_, c)

                # ---- PE: ab GEMM for position pa ----
                if pa is not None:
                    wj = wid(pa)
                    psb = psB.tile([128, 2, BLK], dt.float32, tag="psb")
                    psbs[pa] = psb
                    ht2, slot = hts[pa]
                    for g in range(2):
                        for s in range(3):
                            nc.tensor.matmul(
                                psb[:, g, 0:wj], wab_sb[:, g, s],
                                ht2[:, s, :, slot, 0:wj],
                                start=(s == 0), stop=(s == 2),
                                perf_mode=mybir.MatmulPerfMode.DoubleRow,
                                skip_group_check=True)

                # ---- PE: logits for pl (aT PSUM bank recycled) ----
                if pl is not None:
                    wj = wid(pl)
                    nc.tensor.matmul(psbs[pl][:, 0, 0:wj], wcones_sb[:],
                                     gw_sb, start=True, stop=True,
                                     skip_group_check=True)

            # ---- finalize: reduce slabs (ACT, overlapping the DVE
            # drain), z on DVE, write out ----
            rtrash = constp.tile([128, 6, NRED], dt.float32)
            for c in range(6):
                nc.scalar.activation(rtrash[:, c], pslab[:, c, 0:NRED],
                                     AF.Identity,
                                     accum_out=out_sb[:, c:c + 1])
            nc.vector.tensor_reduce(out_sb[:, 6:7], zslab[:, 0:NB],
                                    mybir.AxisListType.X, ALU.add)
            nc.vector.memset(out_sb[:, 7:8], 0.0)
            nc.sync.dma_start(out_d[:], out_sb[:])

    nc.compile()
    return nc


def get_nc():
    global _cached_nc
    if _cached_nc is None:
        _cached_nc = _build_nc()
    return _cached_nc


def make_inputs(x, W_feat, b_feat, W_a, W_b, W_c):
    """Host-side preprocessing: shard + retile x into the transposed
    DoubleRow layout, prepack weights."""
    x = np.asarray(x, dtype=np.float32)
    xs = x.reshape(N, DIM)
    NP = NB * BLK
    xp = np.zeros((N_CORES, NP, DIM), dtype=np.float32)
    xp[:, :NS, :] = xs.reshape(N_CORES, NS, DIM)
    # xt[core, j, p, s, i, n] = x[core, j*BLK + n, d=(2s+i)*128 + p]
    blocks = xp.reshape(N_CORES, NB, BLK, 3, 2, 128)   # [r, j, n, s, i, p]
    xt_host = np.ascontiguousarray(blocks.transpose(0, 1, 5, 3, 4, 2)) \
        .astype(FP8)

    WT = np.asarray(W_feat, np.float32) * W_SCALE      # [e, d]
    # wt[p, c, s, i, m] = 16*W_feat[c*128+m, (2s+i)*128+p]
    wt_host = np.ascontiguousarray(
        WT.reshape(6, 128, 3, 2, 128).transpose(4, 0, 2, 3, 1)).astype(FP8)

    wab = np.stack([np.asarray(W_a, np.float32),
                    np.asarray(W_b, np.float32)]) * W_SCALE  # [2, k, e]
    # wab[p, g, s, i, m] = 16*W_g[m, (2s+i)*128+p]
    wab_host = np.ascontiguousarray(
        wab.reshape(2, 128, 3, 2, 128).transpose(4, 0, 2, 3, 1)).astype(FP8)

    # bcol[p, c] = 16*b_feat[c*128+p]
    bcol_host = np.ascontiguousarray(
        (np.asarray(b_feat, np.float32).reshape(6, 128).T * W_SCALE)
        .astype(np.float32))

    wcones_host = np.ascontiguousarray(np.tile(
        np.asarray(W_c, np.float32).reshape(D_ATT, 1) / 256.0,
        (1, 128))).astype(BF16)

    mask_host = np.zeros((128, WL), dtype=BF16)
    mask_host[:, :LAST_VALID] = 1

    common = dict(wt=wt_host, wab=wab_host, bcol=bcol_host,
                  wcones=wcones_host, mask=mask_host)
    return [dict(xt=np.ascontiguousarray(xt_host[i]), **common)
            for i in range(N_CORES)]


def _ensure_axon_profile_hook():
    """If someone runs kernel() with BASS_TRACE=1 under axon, the spmd runner
    imports antenv.axon_hooks, which this image lacks; shim it from
    trn_agent_boot so tracing degrades gracefully instead of crashing."""
    try:
        import antenv.axon_hooks  # noqa: F401
        return
    except ImportError:
        pass
    try:
        from trn_agent_boot import trn_boot

        hook = trn_boot._ntff_profile_via_ctypes("/opt/axon/libaxon_pjrt.so")
        mod = types.ModuleType("antenv.axon_hooks")
        mod.get_axon_ntff_profile_hook = lambda: hook
        mod.set_axon_ntff_profile_hook = lambda h: None
        sys.modules["antenv.axon_hooks"] = mod
    except Exception:
        pass


def kernel(x, W_feat, b_feat, W_a, W_b, W_c):
    global last_results
    _ensure_axon_profile_hook()
    nc = get_nc()
    in_maps = make_inputs(x, W_feat, b_feat, W_a, W_b, W_c)
    res = run_bass_kernel_spmd(nc, in_maps, core_ids=list(range(N_CORES)))
    last_results = res
    P = np.zeros(DIM, dtype=np.float64)
    Z = 0.0
    for r in res.results:
        o = np.asarray(r["out"], dtype=np.float64)   # [128, 8]
        P += o[:, 0:6].T.reshape(DIM)
        Z += o[0, 6]
    return (P / W_SCALE / Z).astype(np.float32).reshape(1, DIM)


# revision 36
# speedup vs baseline: 1.2022x; 1.0100x over previous
"""Trainium2 Bass kernel for gated-attention pooling (nn_AttentionGated).

Computation (reference):
    h = relu(x[0] @ W_feat.T + b_feat)        # [N, 768]
    a = relu(h @ W_a.T)                        # [N, 128]
    b = sigmoid(h @ W_b.T)                     # [N, 128]
    logits = (a*b) @ W_c.T                     # [N] -> softmax over N
    out = softmax(logits) @ h                  # [1, 768]

Strategy: shard N=50000 rows over 8 cores (6250 each, padded to 13 blocks of
512 rows). Everything stays in TRANSPOSED [feature-on-partition, row-on-free]
layout, which removes the baseline's PE transposes, bias matmuls and PSUM
transpose-evacuation entirely:

  hT[e, n] = relu(16*W_feat @ x^T + 16*b): 18 fp8 DoubleRow MMs per block
      (stationary = W chunks, moving = x^T chunks). The bias rides the
      PSUM->SBUF evacuations as a per-partition scalar (e is the partition
      axis here): ACT relu(psum + bias) or DVE (psum add bias) max 0, cast
      straight to fp8 (16h fits e4m3 comfortably).
  aT,bT = 256*(W_{a,b} @ h): 6 more DR MMs on the fp8 hT, lagged one block
      so the evacuations stay off the PE critical path.
  sigmoid without ACT-table switches (sigmoid and exp never share an ACT
      table; reloads cost 1.3us each): sigmoid(z) = 0.5*(1+tanh(z/2)), and
      TANH co-resides with Exp/Relu/Identity in the exp_and_others table:
      th = tanh(bT/512) [ACT], v = 0.5*th+0.5 [DVE tensor_scalar, 4x mode],
      gw = relu(aT)*v [DVE scalar_tensor_tensor from PSUM]. ACT then never
      reloads its table.
  logits = one MM with stationary (W_c/256 replicated over M) against gw:
      out[m, n] = logit_n for every m -- BROADCAST logits [128, 512] written
      back into the aT PSUM bank (free after gw consumed it), so
      w = exp(logits) [ACT, accum_out = partial softmax denominator Z for
      free] is already replicated across partitions for the P stage.
  P[e] += sum_n hT[e, n]*w[n]: scalar_tensor_tensor with accum_out on DVE
      (no DVE fast mode exists for this op, so it runs at ~1 col/cycle
      regardless of dtype), batched over 2-block pairs for the steady-state
      positions and unpaired for the last four so the pipeline drain stays
      short; slab columns are reduced once at the end on ACT (Identity with
      accum_out) while the DVE drains.

The host merges the 8 partial (P, Z) pairs: out = sum(P_i)/16 / sum(Z_i).
No on-device collective. W_feat/W_ab/b are pre-scaled x16 on the host to
dodge fp8e4 subnormals; hT is stored as 16h, so aT/bT come out x256 (undone
by the exp scale and by W_c/256) and P comes out x16 (undone on the host).
"""

import sys
import types

import numpy as np
import ml_dtypes

import concourse.bass as bass
import concourse.bacc as bacc
import concourse.mybir as mybir
from concourse import tile
from concourse.bass_utils import run_bass_kernel_spmd

BF16 = ml_dtypes.bfloat16
FP8 = ml_dtypes.float8_e4m3
W_SCALE = 16.0

N_CORES = 8
N = 50000
DIM = 768
D_ATT = 128
NS = N // N_CORES            # 6250 rows per core
BLK = 512                    # rows per block (one full PSUM bank of fp32)
NB = 13                      # blocks per core (6656 rows, last 406 padded)
NRED = 7                     # 6 pairs + 1 solo narrow block
WL = 128                     # width of the narrow last block
LAST_VALID = NS - (NB - 1) * BLK  # 106 valid rows in the last block

_cached_nc = None
last_results = None  # BassKernelResults of the most recent run (for profiling)


def _build_nc():
    AF = mybir.ActivationFunctionType
    ALU = mybir.AluOpType
    dt = mybir.dt

    nc = bacc.Bacc("TRN2", target_bir_lowering=False, debug=False)

    xt_d = nc.dram_tensor("xt", [NB, 128, 3, 2, BLK], dt.float8e4, kind="ExternalInput").ap()
    wt_d = nc.dram_tensor("wt", [128, 6, 3, 2, 128], dt.float8e4, kind="ExternalInput").ap()
    wab_d = nc.dram_tensor("wab", [128, 2, 3, 2, 128], dt.float8e4, kind="ExternalInput").ap()
    bcol_d = nc.dram_tensor("bcol", [128, 6], dt.float32, kind="ExternalInput").ap()
    wcones_d = nc.dram_tensor("wcones", [128, 128], dt.bfloat16, kind="ExternalInput").ap()
    mask_d = nc.dram_tensor("mask", [128, WL], dt.bfloat16, kind="ExternalInput").ap()
    out_d = nc.dram_tensor("out", [128, 8], dt.float32, kind="ExternalOutput").ap()

    # evacuation engine per e-chunk: ACT is cheaper per column and the DVE
    # carries the P accumulations, so ACT takes five of the six.
    EVAC = ["scalar", "vector", "scalar", "scalar", "scalar", "scalar"]

    with tile.TileContext(nc) as tc:
        with (
            tc.tile_pool(name="const", bufs=1) as constp,
            tc.tile_pool(name="xtp", bufs=3) as xtp,
            tc.tile_pool(name="hp", bufs=4) as hp,
            tc.tile_pool(name="tp", bufs=2) as tp,
            tc.tile_pool(name="sgp", bufs=2) as sgp,
            tc.tile_pool(name="gwp", bufs=2) as gwp,
            tc.tile_pool(name="wp", bufs=2) as wp,
            tc.tile_pool(name="trd", bufs=2) as trdp,
            tc.tile_pool(name="psA", bufs=4, space="PSUM") as psA,
            tc.tile_pool(name="psB", bufs=2, space="PSUM") as psB,
        ):
            # --- constants (loaded once) ---
            wt_sb = constp.tile([128, 6, 3, 2, 128], dt.float8e4)
            nc.sync.dma_start(wt_sb[:, 0:3], wt_d[:, 0:3])
            nc.scalar.dma_start(wt_sb[:, 3:6], wt_d[:, 3:6])
            wab_sb = constp.tile([128, 2, 3, 2, 128], dt.float8e4)
            nc.scalar.dma_start(wab_sb[:], wab_d[:])
            bcol_sb = constp.tile([128, 6], dt.float32)
            nc.scalar.dma_start(bcol_sb[:], bcol_d[:])
            wcones_sb = constp.tile([128, 128], dt.bfloat16)
            nc.scalar.dma_start(wcones_sb[:], wcones_d[:])
            mask_sb = constp.tile([128, 128], dt.bfloat16)
            nc.scalar.dma_start(mask_sb[:], mask_d[:])

            # PE warm-up: the PE p-state ramps only while executing, and
            # the first ~9 real matmuls otherwise run 2-3x slow. Dummy
            # DoubleRow MMs on a zeroed tile keep the PE busy during the
            # initial weight/x DMA wait so the real GEMMs start at speed.
            dummy_sb = constp.tile([128, 2, BLK], dt.float8e4)
            nc.gpsimd.memset(dummy_sb[:], 0.0)
            for _w in range(20):
                psa_w = psA.tile([128, BLK], dt.float32, tag="psa")
                nc.tensor.matmul(psa_w[:], dummy_sb[:, :, 0:128], dummy_sb[:],
                                 start=True, stop=True,
                                 perf_mode=mybir.MatmulPerfMode.DoubleRow,
                                 skip_group_check=True)

            zslab = constp.tile([128, 16], dt.float32)
            pslab = constp.tile([128, 6, 12], dt.float32)
            out_sb = constp.tile([128, 8], dt.float32)

            # Block processing order: the narrow block (12) goes FIRST so
            # its serial sigma/w/P chain overlaps the steady state instead
            # of forming a long tail. Positions 1..12 hold blocks 0..11 and
            # pair up as (1,2),(3,4),... for the batched P accumulation.
            ORDER = [NB - 1] + list(range(NB - 1))
            hts = [None] * (NB + 1)    # per position: (pair_tile, slot)
            psbs = [None] * (NB + 1)
            w2s = [None] * (NB + 1)
            wid = lambda pos: WL if pos == 0 else BLK

            def pair_of(pos):
                # pairs (1,2)..(11,12) -> 0..5; solo narrow position 0 -> 6
                return 6 if pos == 0 else (pos - 1) // 2

            def p_acc(pos, c):
                """P accumulation for the pair ending at even position pos
                (chunks summed over both blocks), or the solo position 0."""
                ht2 = hts[pos][0]
                w2, slot = w2s[pos]
                trd = trdp.tile([128, 2, BLK], dt.bfloat16, tag="trd")
                pair = pair_of(pos)
                if pos == 0:
                    trd = trd[:, 0, 0:WL]
                    srcp = ht2[:, c // 2, c % 2, 0, 0:WL]
                    wsrc = w2[:, 0, 0:WL]
                else:
                    trd = trd[:]
                    srcp = ht2[:, c // 2, c % 2, :, :]
                    wsrc = w2[:]
                nc.vector.scalar_tensor_tensor(
                    trd, srcp, 0.0, wsrc, ALU.bypass, ALU.mult,
                    accum_out=pslab[:, c, pair:pair + 1])

            NPOS = NB
            # Stage schedules: position 0 (the narrow block) runs with
            # tighter lags (ab at it=0, sigma at it=1, w at it=2) to pull
            # the whole pipeline one period earlier; the rest use the
            # steady-state lags ab=it-1, sigma=it-2, w=it-3.
            SCHED_AB = {0: 0}
            SCHED_SG = {1: 0}
            SCHED_W = {2: 0}
            for _it in range(2, NPOS + 5):
                if 1 <= _it - 1 < NPOS:
                    SCHED_AB[_it] = _it - 1
            for _it in range(3, NPOS + 5):
                if 1 <= _it - 2 < NPOS:
                    SCHED_SG[_it] = _it - 2
            for _it in range(4, NPOS + 5):
                if 1 <= _it - 3 < NPOS:
                    SCHED_W[_it] = _it - 3
            # P-accumulation triggers: {it: [(pos, chunks)]}
            SCHED_P = {2: [(0, (0, 1, 2))], 3: [(0, (3, 4, 5))]}
            for _pos in range(2, NPOS, 2):   # pairs complete at even pos
                SCHED_P.setdefault(_pos + 3, []).append((_pos, (0, 1, 2)))
                SCHED_P.setdefault(_pos + 4, []).append((_pos, (3, 4, 5)))

            for it in range(NPOS + 5):
                pw = SCHED_W.get(it)
                pl = SCHED_SG.get(it)
                pa = SCHED_AB.get(it)

                # ---- ACT: th(pl) first (feeds the v/gw/logit chain) ----
                if pl is not None:
                    wj = wid(pl)
                    t_sb = tp.tile([128, BLK], dt.bfloat16, tag="t")
                    t_sb = t_sb[:, 0:wj]
                    nc.scalar.activation(t_sb, psbs[pl][:, 1, 0:wj], AF.Tanh,
                                         scale=1.0 / 512.0)

                def w_stage():
                    if pw is None:
                        return
                    blk = ORDER[pw]
                    if pw == 0 or pw % 2 == 1:
                        w2 = wp.tile([128, 2, BLK], dt.bfloat16, tag="w")
                        slot = 0
                    else:
                        w2 = w2s[pw - 1][0]
                        slot = 1
                    w2s[pw] = (w2, slot)
                    if pw == 0:
                        # narrow block: mask the padded rows out of w and
                        # compute Z with an explicit masked reduce.
                        wdst = w2[:, 0, 0:WL]
                        wraw = wp.tile([128, BLK], dt.bfloat16, tag="wr")
                        nc.scalar.activation(wraw[:, 0:WL],
                                             psbs[pw][:, 0, 0:WL], AF.Exp)
                        nc.vector.tensor_tensor(wdst, wraw[:, 0:WL],
                                                mask_sb[:], ALU.mult)
                        nc.vector.tensor_reduce(zslab[:, blk:blk + 1], wdst,
                                                mybir.AxisListType.X, ALU.add)
                    else:
                        wdst = w2[:, slot, :]
                        nc.scalar.activation(wdst, psbs[pw][:, 0], AF.Exp,
                                             accum_out=zslab[:, blk:blk + 1])

                # ---- PE + evacs: h GEMM for position it ----
                if it < NPOS:
                    blk = ORDER[it]
                    wj = wid(it)
                    xt = xtp.tile([128, 3, 2, BLK], dt.float8e4, tag="xt")
                    xt = xt[:, :, :, 0:wj]
                    nc.sync.dma_start(xt[:], xt_d[blk, :, :, :, 0:wj])

                    if it == 0 or it % 2 == 1:
                        ht2 = hp.tile([128, 3, 2, 2, BLK], dt.float8e4,
                                      tag="ht")
                        slot = 0
                    else:
                        ht2 = hts[it - 1][0]
                        slot = 1
                    hts[it] = (ht2, slot)
                    for c in range(6):
                        psa = psA.tile([128, BLK], dt.float32, tag="psa")
                        psa = psa[:, 0:wj]
                        for s in range(3):
                            nc.tensor.matmul(
                                psa[:], wt_sb[:, c, s], xt[:, s],
                                start=(s == 0), stop=(s == 2),
                                perf_mode=mybir.MatmulPerfMode.DoubleRow,
                                skip_group_check=True)
                        dst = ht2[:, c // 2, c % 2, slot, 0:wj]
                        bias = bcol_sb[:, c:c + 1]
                        if EVAC[c] == "scalar":
                            nc.scalar.activation(dst, psa, AF.Relu,
                                                 bias=bias)
                        else:
                            nc.vector.tensor_scalar(dst, psa, bias, 0.0,
                                                    ALU.add, ALU.max)
                        if c == 0:
                            # w emitted after the first evacuation so the
                            # psA rotation is fed promptly
                            w_stage()
                else:
                    w_stage()

                # ---- DVE: sigma chain for pl ----
                if pl is not None:
                    wj = wid(pl)
                    v_sb = sgp.tile([128, BLK], dt.bfloat16, tag="sg")
                    v_sb = v_sb[:, 0:wj]
                    nc.vector.tensor_scalar(v_sb, t_sb, 0.5, 0.5,
                                            ALU.mult, ALU.add)
                    gw_sb = gwp.tile([128, BLK], dt.bfloat16, tag="gw")
                    gw_sb = gw_sb[:, 0:wj]
                    nc.vector.scalar_tensor_tensor(
                        gw_sb, psbs[pl][:, 0, 0:wj], 0.0, v_sb,
                        ALU.max, ALU.mult)

                # ---- DVE: P accumulations per schedule ----
                for pos_, chunks in SCHED_P.get(it, ()):
                    for c in chunks:
                        p_acc(pos_, c)

                # ---- PE: ab GEMM for position pa ----
                if pa is not None:
                    wj = wid(pa)
                    psb = psB.tile([128, 2, BLK], dt.float32, tag="psb")
                    psbs[pa] = psb
                    ht2, slot = hts[pa]
                    for g in range(2):
                        for s in range(3):
                            nc.tensor.matmul(
                                psb[:, g, 0:wj], wab_sb[:, g, s],
                                ht2[:, s, :, slot, 0:wj],
                                start=(s == 0), stop=(s == 2),
                                perf_mode=mybir.MatmulPerfMode.DoubleRow,
                                skip_group_check=True)

                # ---- PE: logits for pl (aT PSUM bank recycled) ----
                if pl is not None:
                    wj = wid(pl)
                    nc.tensor.matmul(psbs[pl][:, 0, 0:wj], wcones_sb[:],
                                     gw_sb, start=True, stop=True,
                                     skip_group_check=True)

            # ---- finalize: reduce slabs (ACT, overlapping the DVE
            # drain), z on DVE, write out ----
            rtrash = constp.tile([128, 6, NRED], dt.float32)
            for c in range(6):
                nc.scalar.activation(rtrash[:, c], pslab[:, c, 0:NRED],
                                     AF.Identity,
                                     accum_out=out_sb[:, c:c + 1])
            nc.vector.tensor_reduce(out_sb[:, 6:7], zslab[:, 0:NB],
                                    mybir.AxisListType.X, ALU.add)
            nc.vector.memset(out_sb[:, 7:8], 0.0)
            nc.sync.dma_start(out_d[:], out_sb[:])

    nc.compile()
    return nc


def get_nc():
    global _cached_nc
    if _cached_nc is None:
        _cached_nc = _build_nc()
    return _cached_nc


def make_inputs(x, W_feat, b_feat, W_a, W_b, W_c):
    """Host-side preprocessing: shard + retile x into the transposed
    DoubleRow layout, prepack weights."""
    x = np.asarray(x, dtype=np.float32)
    xs = x.reshape(N, DIM)
    NP = NB * BLK
    xp = np.zeros((N_CORES, NP, DIM), dtype=np.float32)
    xp[:, :NS, :] = xs.reshape(N_CORES, NS, DIM)
    # xt[core, j, p, s, i, n] = x[core, j*BLK + n, d=(2s+i)*128 + p]
    blocks = xp.reshape(N_CORES, NB, BLK, 3, 2, 128)   # [r, j, n, s, i, p]
    xt_host = np.ascontiguousarray(blocks.transpose(0, 1, 5, 3, 4, 2)) \
        .astype(FP8)

    WT = np.asarray(W_feat, np.float32) * W_SCALE      # [e, d]
    # wt[p, c, s, i, m] = 16*W_feat[c*128+m, (2s+i)*128+p]
    wt_host = np.ascontiguousarray(
        WT.reshape(6, 128, 3, 2, 128).transpose(4, 0, 2, 3, 1)).astype(FP8)

    wab = np.stack([np.asarray(W_a, np.float32),
                    np.asarray(W_b, np.float32)]) * W_SCALE  # [2, k, e]
    # wab[p, g, s, i, m] = 16*W_g[m, (2s+i)*128+p]
    wab_host = np.ascontiguousarray(
        wab.reshape(2, 128, 3, 2, 128).transpose(4, 0, 2, 3, 1)).astype(FP8)

    # bcol[p, c] = 16*b_feat[c*128+p]
    bcol_host = np.ascontiguousarray(
        (np.asarray(b_feat, np.float32).reshape(6, 128).T * W_SCALE)
        .astype(np.float32))

    wcones_host = np.ascontiguousarray(np.tile(
        np.asarray(W_c, np.float32).reshape(D_ATT, 1) / 256.0,
        (1, 128))).astype(BF16)

    mask_host = np.zeros((128, WL), dtype=BF16)
    mask_host[:, :LAST_VALID] = 1

    common = dict(wt=wt_host, wab=wab_host, bcol=bcol_host,
                  wcones=wcones_host, mask=mask_host)
    return [dict(xt=np.ascontiguousarray(xt_host[i]), **common)
            for i in range(N_CORES)]


def _ensure_axon_profile_hook():
    """If someone runs kernel() with BASS_TRACE=1 under axon, the spmd runner
    imports antenv.axon_hooks, which this image lacks; shim it from
    trn_agent_boot so tracing degrades gracefully instead of crashing."""
    try:
        import antenv.axon_hooks  # noqa: F401
        return
    except ImportError:
        pass
    try:
        from trn_agent_boot import trn_boot

        hook = trn_boot._ntff_profile_via_ctypes("/opt/axon/libaxon_pjrt.so")
        mod = types.ModuleType("antenv.axon_hooks")
        mod.get_axon_ntff_profile_hook = lambda: hook
        mod.set_axon_ntff_profile_hook = lambda h: None
        sys.modules["antenv.axon_hooks"] = mod
    except Exception:
        pass


def kernel(x, W_feat, b_feat, W_a, W_b, W_c):
    global last_results
    _ensure_axon_profile_hook()
    nc = get_nc()
    in_maps = make_inputs(x, W_feat, b_feat, W_a, W_b, W_c)
    res = run_bass_kernel_spmd(nc, in_maps, core_ids=list(range(N_CORES)))
    last_results = res
    P = np.zeros(DIM, dtype=np.float64)
    Z = 0.0
    for r in res.results:
        o = np.asarray(r["out"], dtype=np.float64)   # [128, 8]
        P += o[:, 0:6].T.reshape(DIM)
        Z += o[0, 6]
    return (P / W_SCALE / Z).astype(np.float32).reshape(1, DIM)


# revision 38
# speedup vs baseline: 1.2124x; 1.0085x over previous
"""Trainium2 Bass kernel for gated-attention pooling (nn_AttentionGated).

Computation (reference):
    h = relu(x[0] @ W_feat.T + b_feat)        # [N, 768]
    a = relu(h @ W_a.T)                        # [N, 128]
    b = sigmoid(h @ W_b.T)                     # [N, 128]
    logits = (a*b) @ W_c.T                     # [N] -> softmax over N
    out = softmax(logits) @ h                  # [1, 768]

Strategy: shard N=50000 rows over 8 cores (6250 each, padded to 13 blocks of
512 rows). Everything stays in TRANSPOSED [feature-on-partition, row-on-free]
layout, which removes the baseline's PE transposes, bias matmuls and PSUM
transpose-evacuation entirely:

  hT[e, n] = relu(16*W_feat @ x^T + 16*b): 18 fp8 DoubleRow MMs per block
      (stationary = W chunks, moving = x^T chunks). The bias rides the
      PSUM->SBUF evacuations as a per-partition scalar (e is the partition
      axis here): ACT relu(psum + bias) or DVE (psum add bias) max 0, cast
      straight to fp8 (16h fits e4m3 comfortably).
  aT,bT = 256*(W_{a,b} @ h): 6 more DR MMs on the fp8 hT, lagged one block
      so the evacuations stay off the PE critical path.
  sigmoid without ACT-table switches (sigmoid and exp never share an ACT
      table; reloads cost 1.3us each): sigmoid(z) = 0.5*(1+tanh(z/2)), and
      TANH co-resides with Exp/Relu/Identity in the exp_and_others table:
      th = tanh(bT/512) [ACT], v = 0.5*th+0.5 [DVE tensor_scalar, 4x mode],
      gw = relu(aT)*v [DVE scalar_tensor_tensor from PSUM]. ACT then never
      reloads its table.
  logits = one MM with stationary (W_c/256 replicated over M) against gw:
      out[m, n] = logit_n for every m -- BROADCAST logits [128, 512] written
      back into the aT PSUM bank (free after gw consumed it), so
      w = exp(logits) [ACT, accum_out = partial softmax denominator Z for
      free] is already replicated across partitions for the P stage.
  P[e] += sum_n hT[e, n]*w[n]: scalar_tensor_tensor with accum_out on DVE
      (no DVE fast mode exists for this op, so it runs at ~1 col/cycle
      regardless of dtype), batched over 2-block pairs for the steady-state
      positions and unpaired for the last four so the pipeline drain stays
      short; slab columns are reduced once at the end on ACT (Identity with
      accum_out) while the DVE drains.

The host merges the 8 partial (P, Z) pairs: out = sum(P_i)/16 / sum(Z_i).
No on-device collective. W_feat/W_ab/b are pre-scaled x16 on the host to
dodge fp8e4 subnormals; hT is stored as 16h, so aT/bT come out x256 (undone
by the exp scale and by W_c/256) and P comes out x16 (undone on the host).
"""

import sys
import types

import numpy as np
import ml_dtypes

import concourse.bass as bass
import concourse.bacc as bacc
import concourse.mybir as mybir
from concourse import tile
from concourse.bass_utils import run_bass_kernel_spmd

BF16 = ml_dtypes.bfloat16
FP8 = ml_dtypes.float8_e4m3
W_SCALE = 16.0

N_CORES = 8
N = 50000
DIM = 768
D_ATT = 128
NS = N // N_CORES            # 6250 rows per core
BLK = 512                    # rows per block (one full PSUM bank of fp32)
NB = 13                      # blocks per core (6656 rows, last 406 padded)
NRED = 7                     # 6 pairs + 1 solo narrow block
WL = 128                     # width of the narrow last block
LAST_VALID = NS - (NB - 1) * BLK  # 106 valid rows in the last block

_cached_nc = None
last_results = None  # BassKernelResults of the most recent run (for profiling)


def _build_nc():
    AF = mybir.ActivationFunctionType
    ALU = mybir.AluOpType
    dt = mybir.dt

    nc = bacc.Bacc("TRN2", target_bir_lowering=False, debug=False)

    xt_d = nc.dram_tensor("xt", [NB, 128, 3, 2, BLK], dt.float8e4, kind="ExternalInput").ap()
    wt_d = nc.dram_tensor("wt", [128, 6, 3, 2, 128], dt.float8e4, kind="ExternalInput").ap()
    wab_d = nc.dram_tensor("wab", [128, 2, 3, 2, 128], dt.float8e4, kind="ExternalInput").ap()
    bcol_d = nc.dram_tensor("bcol", [128, 6], dt.float32, kind="ExternalInput").ap()
    wcones_d = nc.dram_tensor("wcones", [128, 128], dt.bfloat16, kind="ExternalInput").ap()
    mask_d = nc.dram_tensor("mask", [128, WL], dt.bfloat16, kind="ExternalInput").ap()
    out_d = nc.dram_tensor("out", [128, 8], dt.float32, kind="ExternalOutput").ap()

    # evacuation engine per e-chunk: ACT is cheaper per column and the DVE
    # carries the P accumulations, so ACT takes five of the six.
    EVAC = ["scalar", "vector", "scalar", "scalar", "scalar", "scalar"]

    with tile.TileContext(nc) as tc:
        with (
            tc.tile_pool(name="const", bufs=1) as constp,
            tc.tile_pool(name="xtp", bufs=3) as xtp,
            tc.tile_pool(name="hp", bufs=4) as hp,
            tc.tile_pool(name="tp", bufs=2) as tp,
            tc.tile_pool(name="sgp", bufs=2) as sgp,
            tc.tile_pool(name="gwp", bufs=2) as gwp,
            tc.tile_pool(name="wp", bufs=2) as wp,
            tc.tile_pool(name="trd", bufs=2) as trdp,
            tc.tile_pool(name="psA", bufs=4, space="PSUM") as psA,
            tc.tile_pool(name="psB", bufs=2, space="PSUM") as psB,
        ):
            # --- constants (loaded once) ---
            wt_sb = constp.tile([128, 6, 3, 2, 128], dt.float8e4)
            nc.sync.dma_start(wt_sb[:, 0:3], wt_d[:, 0:3])
            nc.scalar.dma_start(wt_sb[:, 3:6], wt_d[:, 3:6])
            wab_sb = constp.tile([128, 2, 3, 2, 128], dt.float8e4)
            nc.scalar.dma_start(wab_sb[:], wab_d[:])
            bcol_sb = constp.tile([128, 6], dt.float32)
            nc.scalar.dma_start(bcol_sb[:], bcol_d[:])
            wcones_sb = constp.tile([128, 128], dt.bfloat16)
            nc.scalar.dma_start(wcones_sb[:], wcones_d[:])
            mask_sb = constp.tile([128, 128], dt.bfloat16)
            nc.scalar.dma_start(mask_sb[:], mask_d[:])

            # PE warm-up: the PE p-state ramps only while executing, and
            # the first ~9 real matmuls otherwise run 2-3x slow. Dummy
            # DoubleRow MMs on a zeroed tile keep the PE busy during the
            # initial weight/x DMA wait so the real GEMMs start at speed.
            dummy_sb = constp.tile([128, 2, BLK], dt.float8e4)
            nc.gpsimd.memset(dummy_sb[:], 0.0)
            half_sb = constp.tile([128, 1], dt.float32)
            nc.vector.memset(half_sb[:], 0.5)
            for _w in range(20):
                psa_w = psA.tile([128, BLK], dt.float32, tag="psa")
                nc.tensor.matmul(psa_w[:], dummy_sb[:, :, 0:128], dummy_sb[:],
                                 start=True, stop=True,
                                 perf_mode=mybir.MatmulPerfMode.DoubleRow,
                                 skip_group_check=True)

            zslab = constp.tile([128, 16], dt.float32)
            pslab = constp.tile([128, 6, 12], dt.float32)
            out_sb = constp.tile([128, 8], dt.float32)

            # Block processing order: the narrow block (12) goes FIRST so
            # its serial sigma/w/P chain overlaps the steady state instead
            # of forming a long tail. Positions 1..12 hold blocks 0..11 and
            # pair up as (1,2),(3,4),... for the batched P accumulation.
            ORDER = [NB - 1] + list(range(NB - 1))
            hts = [None] * (NB + 1)    # per position: (pair_tile, slot)
            psbs = [None] * (NB + 1)
            w2s = [None] * (NB + 1)
            wid = lambda pos: WL if pos == 0 else BLK

            def pair_of(pos):
                # pairs (1,2)..(11,12) -> 0..5; solo narrow position 0 -> 6
                return 6 if pos == 0 else (pos - 1) // 2

            def p_acc(pos, c):
                """P accumulation for the pair ending at even position pos
                (chunks summed over both blocks), or the solo position 0."""
                ht2 = hts[pos][0]
                w2, slot = w2s[pos]
                trd = trdp.tile([128, 2, BLK], dt.bfloat16, tag="trd")
                pair = pair_of(pos)
                if pos == 0:
                    trd = trd[:, 0, 0:WL]
                    srcp = ht2[:, c // 2, c % 2, 0, 0:WL]
                    wsrc = w2[:, 0, 0:WL]
                else:
                    trd = trd[:]
                    srcp = ht2[:, c // 2, c % 2, :, :]
                    wsrc = w2[:]
                nc.vector.scalar_tensor_tensor(
                    trd, srcp, 0.0, wsrc, ALU.bypass, ALU.mult,
                    accum_out=pslab[:, c, pair:pair + 1])

            NPOS = NB
            # Stage schedules: position 0 (the narrow block) runs with
            # tighter lags (ab at it=0, sigma at it=1, w at it=2) to pull
            # the whole pipeline one period earlier; the rest use the
            # steady-state lags ab=it-1, sigma=it-2, w=it-3.
            SCHED_AB = {0: 0}
            SCHED_SG = {1: 0}
            SCHED_W = {2: 0}
            for _it in range(2, NPOS + 5):
                if 1 <= _it - 1 < NPOS:
                    SCHED_AB[_it] = _it - 1
            for _it in range(3, NPOS + 5):
                if 1 <= _it - 2 < NPOS:
                    SCHED_SG[_it] = _it - 2
            for _it in range(4, NPOS + 5):
                if 1 <= _it - 3 < NPOS:
                    SCHED_W[_it] = _it - 3
            # P-accumulation triggers: {it: [(pos, chunks)]}
            SCHED_P = {2: [(0, (0, 1, 2))], 3: [(0, (3, 4, 5))]}
            for _pos in range(2, NPOS, 2):   # pairs complete at even pos
                SCHED_P.setdefault(_pos + 3, []).append((_pos, (0, 1, 2)))
                SCHED_P.setdefault(_pos + 4, []).append((_pos, (3, 4, 5)))

            for it in range(NPOS + 5):
                pw = SCHED_W.get(it)
                pl = SCHED_SG.get(it)
                pa = SCHED_AB.get(it)

                # ---- ACT: th(pl) first (feeds the v/gw/logit chain) ----
                if pl is not None:
                    wj = wid(pl)
                    t_sb = tp.tile([128, BLK], dt.bfloat16, tag="t")
                    t_sb = t_sb[:, 0:wj]
                    nc.scalar.activation(t_sb, psbs[pl][:, 1, 0:wj], AF.Tanh,
                                         scale=1.0 / 512.0)

                def w_stage():
                    if pw is None:
                        return
                    blk = ORDER[pw]
                    if pw == 0 or pw % 2 == 1:
                        w2 = wp.tile([128, 2, BLK], dt.bfloat16, tag="w")
                        slot = 0
                    else:
                        w2 = w2s[pw - 1][0]
                        slot = 1
                    w2s[pw] = (w2, slot)
                    if pw == 0:
                        # narrow block: mask the padded rows out of w and
                        # compute Z with an explicit masked reduce.
                        wdst = w2[:, 0, 0:WL]
                        wraw = wp.tile([128, BLK], dt.bfloat16, tag="wr")
                        nc.scalar.activation(wraw[:, 0:WL],
                                             psbs[pw][:, 0, 0:WL], AF.Exp)
                        nc.vector.tensor_tensor(wdst, wraw[:, 0:WL],
                                                mask_sb[:], ALU.mult)
                        nc.vector.tensor_reduce(zslab[:, blk:blk + 1], wdst,
                                                mybir.AxisListType.X, ALU.add)
                    else:
                        wdst = w2[:, slot, :]
                        nc.scalar.activation(wdst, psbs[pw][:, 0], AF.Exp,
                                             accum_out=zslab[:, blk:blk + 1])

                # ---- PE + evacs: h GEMM for position it ----
                if it < NPOS:
                    blk = ORDER[it]
                    wj = wid(it)
                    xt = xtp.tile([128, 3, 2, BLK], dt.float8e4, tag="xt")
                    xt = xt[:, :, :, 0:wj]
                    nc.sync.dma_start(xt[:], xt_d[blk, :, :, :, 0:wj])

                    if it == 0 or it % 2 == 1:
                        ht2 = hp.tile([128, 3, 2, 2, BLK], dt.float8e4,
                                      tag="ht")
                        slot = 0
                    else:
                        ht2 = hts[it - 1][0]
                        slot = 1
                    hts[it] = (ht2, slot)
                    for c in range(6):
                        psa = psA.tile([128, BLK], dt.float32, tag="psa")
                        psa = psa[:, 0:wj]
                        for s in range(3):
                            nc.tensor.matmul(
                                psa[:], wt_sb[:, c, s], xt[:, s],
                                start=(s == 0), stop=(s == 2),
                                perf_mode=mybir.MatmulPerfMode.DoubleRow,
                                skip_group_check=True)
                        dst = ht2[:, c // 2, c % 2, slot, 0:wj]
                        bias = bcol_sb[:, c:c + 1]
                        if EVAC[c] == "scalar":
                            nc.scalar.activation(dst, psa, AF.Relu,
                                                 bias=bias)
                        else:
                            nc.vector.tensor_scalar(dst, psa, bias, 0.0,
                                                    ALU.add, ALU.max)
                        if c == 0:
                            # w emitted after the first evacuation so the
                            # psA rotation is fed promptly
                            w_stage()
                else:
                    w_stage()

                # ---- sigma chain for pl (v on ACT during the drain,
                # where DVE is the bottleneck and ACT is mostly idle) ----
                if pl is not None:
                    wj = wid(pl)
                    v_sb = sgp.tile([128, BLK], dt.bfloat16, tag="sg")
                    v_sb = v_sb[:, 0:wj]
                    if pl >= NPOS - 2:
                        nc.scalar.activation(v_sb, t_sb, AF.Identity,
                                             bias=half_sb[:], scale=0.5)
                    else:
                        nc.vector.tensor_scalar(v_sb, t_sb, 0.5, 0.5,
                                                ALU.mult, ALU.add)
                    gw_sb = gwp.tile([128, BLK], dt.bfloat16, tag="gw")
                    gw_sb = gw_sb[:, 0:wj]
                    nc.vector.scalar_tensor_tensor(
                        gw_sb, psbs[pl][:, 0, 0:wj], 0.0, v_sb,
                        ALU.max, ALU.mult)

                # ---- DVE: P accumulations per schedule ----
                for pos_, chunks in SCHED_P.get(it, ()):
                    for c in chunks:
                        p_acc(pos_, c)

                # ---- PE: ab GEMM for position pa ----
                if pa is not None:
                    wj = wid(pa)
                    psb = psB.tile([128, 2, BLK], dt.float32, tag="psb")
                    psbs[pa] = psb
                    ht2, slot = hts[pa]
                    for g in range(2):
                        for s in range(3):
                            nc.tensor.matmul(
                                psb[:, g, 0:wj], wab_sb[:, g, s],
                                ht2[:, s, :, slot, 0:wj],
                                start=(s == 0), stop=(s == 2),
                                perf_mode=mybir.MatmulPerfMode.DoubleRow,
                                skip_group_check=True)

                # ---- PE: logits for pl (aT PSUM bank recycled) ----
                if pl is not None:
                    wj = wid(pl)
                    nc.tensor.matmul(psbs[pl][:, 0, 0:wj], wcones_sb[:],
                                     gw_sb, start=True, stop=True,
                                     skip_group_check=True)

            # ---- finalize: reduce slabs (ACT, overlapping the DVE
            # drain), z on DVE, write out ----
            rtrash = constp.tile([128, 6, NRED], dt.float32)
            for c in range(6):
                nc.scalar.activation(rtrash[:, c], pslab[:, c, 0:NRED],
                                     AF.Identity,
                                     accum_out=out_sb[:, c:c + 1])
            nc.vector.tensor_reduce(out_sb[:, 6:7], zslab[:, 0:NB],
                                    mybir.AxisListType.X, ALU.add)
            nc.vector.memset(out_sb[:, 7:8], 0.0)
            nc.sync.dma_start(out_d[:], out_sb[:])

    nc.compile()
    return nc


def get_nc():
    global _cached_nc
    if _cached_nc is None:
        _cached_nc = _build_nc()
    return _cached_nc


def make_inputs(x, W_feat, b_feat, W_a, W_b, W_c):
    """Host-side preprocessing: shard + retile x into the transposed
    DoubleRow layout, prepack weights."""
    x = np.asarray(x, dtype=np.float32)
    xs = x.reshape(N, DIM)
    NP = NB * BLK
    xp = np.zeros((N_CORES, NP, DIM), dtype=np.float32)
    xp[:, :NS, :] = xs.reshape(N_CORES, NS, DIM)
    # xt[core, j, p, s, i, n] = x[core, j*BLK + n, d=(2s+i)*128 + p]
    blocks = xp.reshape(N_CORES, NB, BLK, 3, 2, 128)   # [r, j, n, s, i, p]
    xt_host = np.ascontiguousarray(blocks.transpose(0, 1, 5, 3, 4, 2)) \
        .astype(FP8)

    WT = np.asarray(W_feat, np.float32) * W_SCALE      # [e, d]
    # wt[p, c, s, i, m] = 16*W_feat[c*128+m, (2s+i)*128+p]
    wt_host = np.ascontiguousarray(
        WT.reshape(6, 128, 3, 2, 128).transpose(4, 0, 2, 3, 1)).astype(FP8)

    wab = np.stack([np.asarray(W_a, np.float32),
                    np.asarray(W_b, np.float32)]) * W_SCALE  # [2, k, e]
    # wab[p, g, s, i, m] = 16*W_g[m, (2s+i)*128+p]
    wab_host = np.ascontiguousarray(
        wab.reshape(2, 128, 3, 2, 128).transpose(4, 0, 2, 3, 1)).astype(FP8)

    # bcol[p, c] = 16*b_feat[c*128+p]
    bcol_host = np.ascontiguousarray(
        (np.asarray(b_feat, np.float32).reshape(6, 128).T * W_SCALE)
        .astype(np.float32))

    wcones_host = np.ascontiguousarray(np.tile(
        np.asarray(W_c, np.float32).reshape(D_ATT, 1) / 256.0,
        (1, 128))).astype(BF16)

    mask_host = np.zeros((128, WL), dtype=BF16)
    mask_host[:, :LAST_VALID] = 1

    common = dict(wt=wt_host, wab=wab_host, bcol=bcol_host,
                  wcones=wcones_host, mask=mask_host)
    return [dict(xt=np.ascontiguousarray(xt_host[i]), **common)
            for i in range(N_CORES)]


def _ensure_axon_profile_hook():
    """If someone runs kernel() with BASS_TRACE=1 under axon, the spmd runner
    imports antenv.axon_hooks, which this image lacks; shim it from
    trn_agent_boot so tracing degrades gracefully instead of crashing."""
    try:
        import antenv.axon_hooks  # noqa: F401
        return
    except ImportError:
        pass
    try:
        from trn_agent_boot import trn_boot

        hook = trn_boot._ntff_profile_via_ctypes("/opt/axon/libaxon_pjrt.so")
        mod = types.ModuleType("antenv.axon_hooks")
        mod.get_axon_ntff_profile_hook = lambda: hook
        mod.set_axon_ntff_profile_hook = lambda h: None
        sys.modules["antenv.axon_hooks"] = mod
    except Exception:
        pass


def kernel(x, W_feat, b_feat, W_a, W_b, W_c):
    global last_results
    _ensure_axon_profile_hook()
    nc = get_nc()
    in_maps = make_inputs(x, W_feat, b_feat, W_a, W_b, W_c)
    res = run_bass_kernel_spmd(nc, in_maps, core_ids=list(range(N_CORES)))
    last_results = res
    P = np.zeros(DIM, dtype=np.float64)
    Z = 0.0
    for r in res.results:
        o = np.asarray(r["out"], dtype=np.float64)   # [128, 8]
        P += o[:, 0:6].T.reshape(DIM)
        Z += o[0, 6]
    return (P / W_SCALE / Z).astype(np.float32).reshape(1, DIM)
